# revision 6
# baseline (speedup 1.0000x reference)
"""Trainium2 Bass kernel for nn_DiscreteTimeS4.

Model (reference):
    x_proj = relu(x_seq @ W_in^T + b_in)                  # [B, T, P]
    h_t = a * h_{t-1} + x_proj_t @ B ;  y_t = h_t @ C     # diagonal SSM scan
    out = y @ W_out^T + b_out                             # [B, T, OUT]

Key transform: |a| <= sqrt(2/H) ~ 0.09, so a^k decays below the fp16
operand noise floor within a few steps.  The scan is therefore (to fp32
precision) a short causal convolution over time, and W_out folds into
the conv matrices:
    out_t = sum_k x_proj_{t-k} @ F_k + b_out,
    F_k = B @ diag(a^k) @ C @ W_out^T          # [P, OUT], host-folded fp64

Device pipeline per batch-row PAIR (rows 2rp, 2rp+1 share the PE array):
    load:    one [128, T] fp16 DMA (row j of the pair in partitions
             64j:64j+64) -- x is loaded exactly once
    stage 1: ps1 = W_in @ x_chunk per (row, chunk); the two rows run
             concurrently in disjoint PE row groups (tile_position).
             Chunks land pairwise in one 2-bank PSUM tile.
    relu:    xproj[j][:, PAD+cp*1024:...] = relu(ps1 + b_in) -> fp16 in
             one [128, 1024] op, alternating DVE / ACT.  xproj is ONE
             [128, PAD+T] tile per row, so the lagged stage-2 windows
             are free SBUF column offsets (zero pad head, memset once).
    stage 2: pso[half*64:, :] += F_k^T @ xproj(shift k) -- the two
             chunks of a pair run concurrently in disjoint PE column
             groups; n_lags PSUM-accumulated matmuls per (row, pair)
    cast:    pso fp32 -> out_sb fp16 (DVE/ACT alternating), laid out
             [half*64+o, j*1024 + p*512 + t]
    store:   one [128, 2048] fp16 DMA per row pair (512 KB contiguous)
Final unshuffle ([half, o, j, p, t] -> [b, t, o]) + fp32 cast happen on
the host; b_out is added on the host (all-zero for this model).

Sharding: data-parallel over batch, 8 NeuronCores, B=64 -> 8 per core.
"""

import os
import sys

for _p in ("/opt/trn_rl_repo", "/root/.axon_site/_ro/trn_rl_repo"):
    if os.path.isdir(_p) and _p not in sys.path:
        sys.path.append(_p)

import numpy as np

import concourse.bacc as bacc
import concourse.mybir as mybir
from concourse.bass_utils import run_bass_kernel_spmd
from concourse.tile import TileContext

BATCH, T, IN, P, H, OUT = 64, 2048, 64, 128, 256, 64
NCORES = 8
BL = BATCH // NCORES          # batch rows per core
NRP = BL // 2                 # row pairs per core
CHUNK = 512                   # time chunk (one fp32 PSUM bank)
NCHUNK = T // CHUNK           # 4
NPAIR = NCHUNK // 2           # chunk pairs per row

# a^k truncation threshold (relative to output scale).  3e-3 gives
# n_lags=3 for this model's |a|max ~ 0.088; truncation error ~7e-4 of
# output scale, well under the 2e-2 gate and comparable to fp16 noise.
LAG_TRUNC_THRESH = 3e-3

F32 = mybir.dt.float32
F16 = mybir.dt.float16

_programs = {}                # n_lags -> finalized Bacc program


def _build(n_lags: int):
    nc = bacc.Bacc("TRN2", target_bir_lowering=False, num_devices=NCORES)

    PAD = n_lags - 1

    x = nc.declare_dram_parameter("x", [NRP, 2 * IN, T], F16, isOutput=False)
    # [P, n_lags*OUT] conv matrices, host-prepacked
    wfold = nc.declare_dram_parameter("wfold", [P, n_lags * OUT], F16,
                                      isOutput=False)
    # W_in^T duplicated into both partition halves for row-group packing
    w_inT = nc.declare_dram_parameter("w_inT", [2 * IN, P], F16, isOutput=False)
    b_in = nc.declare_dram_parameter("b_in", [P, 1], F32, isOutput=False)
    out = nc.declare_dram_parameter("out", [NRP, 2 * OUT, T], F16,
                                    isOutput=True)

    with TileContext(nc) as tc:
        with (
            tc.tile_pool(name="wpool", bufs=1) as wpool,
            tc.tile_pool(name="xin", bufs=3) as xin_pool,
            tc.tile_pool(name="xproj", bufs=4) as xp_pool,
            tc.tile_pool(name="osb", bufs=2) as osb_pool,
            tc.tile_pool(name="ps1", bufs=3, space="PSUM") as ps1_pool,
            tc.tile_pool(name="pso", bufs=2, space="PSUM") as pso_pool,
        ):
            # ---- load weights once (already fp16/fp32 from host) ----
            fk = wpool.tile([P, n_lags * OUT], F16)
            nc.sync.dma_start(out=fk[:], in_=wfold[:])
            wi = wpool.tile([2 * IN, P], F16)
            nc.sync.dma_start(out=wi[:], in_=w_inT[:])
            bi = wpool.tile([P, 1], F32)
            nc.sync.dma_start(out=bi[:], in_=b_in[:])

            ew = [0]          # elementwise op toggle (DVE <-> ACT)

            def relu_op(dst, src):
                if ew[0] % 2 == 0:
                    nc.vector.tensor_scalar(
                        out=dst, in0=src, scalar1=bi[:], scalar2=0.0,
                        op0=mybir.AluOpType.add, op1=mybir.AluOpType.max,
                    )
                else:
                    nc.scalar.activation(
                        out=dst, in_=src,
                        func=mybir.ActivationFunctionType.Relu, bias=bi[:],
                    )
                ew[0] += 1

            def cast_op(dst, src):
                if ew[0] % 2 == 0:
                    nc.vector.tensor_copy(out=dst, in_=src)
                else:
                    nc.scalar.activation(
                        out=dst, in_=src,
                        func=mybir.ActivationFunctionType.Copy,
                    )
                ew[0] += 1

            def stage1(rp):
                """load + input projection for both rows of the pair;
                returns the two [P, PAD+T] fp16 xproj tiles."""
                xTr = xin_pool.tile([2 * IN, T], F16, tag="xTr")
                nc.sync.dma_start(out=xTr[:], in_=x[rp])
                xps = []
                for j in range(2):
                    xp = xp_pool.tile([P, PAD + T], F16, tag="xp")
                    nc.gpsimd.memset(xp[:, 0:PAD], 0.0)
                    xps.append(xp)
                for cp in range(NCHUNK // 2):          # chunk pairs
                    ps1s = [ps1_pool.tile([P, 2 * CHUNK], F32, tag="ps1",
                                          name=f"ps1_{rp}_{cp}_{jj}")
                            for jj in range(2)]
                    for h in range(2):                 # chunk within pair
                        c = 2 * cp + h
                        for j in range(2):             # row in row pair
                            nc.tensor.matmul(
                                ps1s[j][:, h * CHUNK:(h + 1) * CHUNK],
                                wi[j * IN:(j + 1) * IN, :],
                                xTr[j * IN:(j + 1) * IN,
                                    c * CHUNK:(c + 1) * CHUNK],
                                start=True, stop=True,
                                tile_position=(j * IN, 0),
                            )
                    for j in range(2):
                        relu_op(
                            xps[j][:, PAD + 2 * cp * CHUNK:
                                   PAD + 2 * (cp + 1) * CHUNK],
                            ps1s[j][:],
                        )
                return xps

            def stage2(rp, xps):
                """fused conv for both rows -> fp16 out_sb -> one DMA."""
                osb = osb_pool.tile([2 * OUT, T], F16, tag="osb")
                for j in range(2):
                    for p in range(NPAIR):
                        pso = pso_pool.tile([2 * OUT, CHUNK], F32, tag="pso")
                        for k in range(n_lags):
                            for half in range(2):
                                base = PAD + (2 * p + half) * CHUNK - k
                                nc.tensor.matmul(
                                    pso[half * OUT:(half + 1) * OUT, :],
                                    fk[:, k * OUT:(k + 1) * OUT],
                                    xps[j][:, base: base + CHUNK],
                                    start=(k == 0), stop=(k == n_lags - 1),
                                    tile_position=(0, half * OUT),
                                )
                        cast_op(
                            osb[:, (2 * j + p) * CHUNK:
                                (2 * j + p + 1) * CHUNK],
                            pso[:],
                        )
                nc.sync.dma_start(out=out[rp], in_=osb[:])

            # ---- software-pipelined main loop over row pairs ----
            # stage2(rp) is emitted after stage1(rp+1) so the PE never
            # waits on a relu: it always has the next pair's projection
            # matmuls to chew on.
            pend = []
            for rp in range(NRP):
                pend.append((rp, stage1(rp)))
                if len(pend) > 1:
                    prp, pxps = pend.pop(0)
                    stage2(prp, pxps)
            for prp, pxps in pend:
                stage2(prp, pxps)

    nc.finalize()
    return nc


def _n_lags(a: np.ndarray) -> int:
    amax = float(np.abs(a).max())
    if amax >= 1.0:
        return 16
    if amax <= 0.0:
        return 2
    k = int(np.ceil(np.log(LAG_TRUNC_THRESH) / np.log(amax)))
    return max(2, min(16, k))


def _prepare(x_seq, a, B, C, W_in, b_in, W_out, b_out):
    """Host-side folding + per-core input maps."""
    n_lags = _n_lags(a)
    a64 = a.astype(np.float64)
    B64 = B.astype(np.float64)
    CW64 = C.astype(np.float64) @ W_out.T.astype(np.float64)   # [H, OUT]
    fks = np.concatenate(
        [(B64 * (a64 ** k)[None, :]) @ CW64 for k in range(n_lags)],
        axis=1,
    ).astype(np.float16)                                       # [P, K*OUT]
    wiT = W_in.T.astype(np.float16)
    shared = {
        "wfold": np.ascontiguousarray(fks),
        "w_inT": np.ascontiguousarray(np.vstack([wiT, wiT])),
        "b_in": np.ascontiguousarray(b_in.astype(np.float32).reshape(P, 1)),
    }
    xT = np.swapaxes(x_seq, 1, 2).astype(np.float16)           # [B, IN, T]
    xT = np.ascontiguousarray(xT).reshape(NCORES, NRP, 2 * IN, T)
    in_maps = []
    for c in range(NCORES):
        m = dict(shared)
        m["x"] = xT[c]
        in_maps.append(m)
    return n_lags, in_maps


def _decode_out(res):
    """[NRP, 2*OUT, T] fp16 per core -> [BATCH, T, OUT] fp32."""
    arr = np.stack([res[c]["out"] for c in range(NCORES)])
    # [core, rp, half, o, j, p, t]
    arr = arr.reshape(NCORES, NRP, 2, OUT, 2, NPAIR, CHUNK)
    # -> [core, rp, j, p, half, t, o]  (time = (2p+half)*CHUNK + t)
    arr = arr.transpose(0, 1, 4, 5, 2, 6, 3)
    return arr.reshape(BATCH, T, OUT).astype(np.float32)


def get_program(n_lags: int, reps: int = 1):
    key = n_lags
    if key not in _programs:
        _programs[key] = _build(n_lags)
    return _programs[key]


def kernel(x_seq, a, B, C, W_in, b_in, W_out, b_out):
    n_lags, in_maps = _prepare(x_seq, a, B, C, W_in, b_in, W_out, b_out)
    nc = get_program(n_lags)
    res = run_bass_kernel_spmd(nc, in_maps, list(range(NCORES)))
    out = _decode_out(res.results)
    if np.any(b_out):
        out = out + b_out.astype(np.float32).reshape(1, 1, OUT)
    return out
